# revision 5
# baseline (speedup 1.0000x reference)
"""Trainium2 Bass kernel for nn_BertEncoder_403726926494.

Reference computation (per batch element):
  - ragged sentence extraction from hidden_states, masked-softmax attention
    pooling per sentence with W_doc            -> doc_pooled [B, D, H]
  - query extraction (rows 1..32), masked-softmax pooling with W_query
    broadcast over D                           -> q_bcast   [B, D, H]

Device strategy (SPMD, one program on 8 cores, 8 batch elements per core):
  - Per core-slot, DMA only the used row-span of hidden_states into SBUF
    (slots are assigned from a global sort of spans so the per-slot span is
    a static program constant shared by all cores).
  - Per-token scores s[t] = x_t . W via ONE fused DVE tensor_tensor_reduce
    per 128-row chunk (multiply by a W-broadcast tile, reduce over H).
  - softmax without max-subtraction (scores are O(1)):
        alphaU[t] = exp(s[t]);  num = SelT^T @ (alphaU * X);  den = SelT^T @ alphaU
    where SelT[t, j] in {0,1} marks token t belonging to sentence j.  SelT is
    built host-side from the int32 length inputs and shipped as input data, so
    the pooling is a plain PE matmul with lhsT = SelT * alphaU (a per-partition
    scale op) and PSUM accumulation over chunks.
  - out[j] = num[j] * 1/(den[j] + eps)  (eps keeps empty sentences at 0).
  - The query path packs 4 examples x 32 query rows onto 128 partitions and
    runs the same scheme; masking enters as an additive -1e30 bias in the exp.
    q_pooled is returned per example and broadcast over D on the host.
  - b_doc / b_query shift every score in a softmax segment equally, so they
    cancel and are ignored.
"""

import numpy as np

B, L, H = 64, 512, 768
D, S, Q = 16, 64, 32
NCORES = 8
SLOTS = 8
NEG_BIAS = -1.0e30
DEN_EPS = 1.0e-30

_compiled: dict = {}


def _slot_geometry(slot_spans):
    nts = [(sp + 127) // 128 for sp in slot_spans]
    rems = [sp - 128 * (nt - 1) for sp, nt in zip(slot_spans, nts)]
    coffs = [0]
    for nt in nts:
        coffs.append(coffs[-1] + nt)
    return nts, rems, coffs


def _build(slot_spans):
    """Build + compile the SPMD Bass program for the given per-slot spans."""
    from contextlib import ExitStack

    import concourse.bacc as bacc
    import concourse.tile as tile
    from concourse import mybir

    f32 = mybir.dt.float32
    MULT = mybir.AluOpType.mult
    ADD = mybir.AluOpType.add
    EXP = mybir.ActivationFunctionType.Exp
    COPY = mybir.ActivationFunctionType.Copy

    nts, rems, coffs = _slot_geometry(slot_spans)
    ntsum = coffs[-1]

    nc = bacc.Bacc(
        "TRN2", target_bir_lowering=False, debug=False, num_devices=NCORES
    )
    hidden = nc.dram_tensor("hidden", [SLOTS, L, H], f32, kind="ExternalInput").ap()
    wd = nc.dram_tensor("wd", [1, H], f32, kind="ExternalInput").ap()
    wq = nc.dram_tensor("wq", [1, H], f32, kind="ExternalInput").ap()
    selt = nc.dram_tensor("selt", [128, ntsum, D], f32, kind="ExternalInput").ap()
    qbias = nc.dram_tensor("qbias", [128, 2], f32, kind="ExternalInput").ap()
    doc_out = nc.dram_tensor("doc_out", [SLOTS, D, H], f32, kind="ExternalOutput").ap()
    q_out = nc.dram_tensor("q_out", [SLOTS, H], f32, kind="ExternalOutput").ap()

    with tile.TileContext(nc) as tc, ExitStack() as ctx:
        const = ctx.enter_context(tc.tile_pool(name="const", bufs=1))

        ones_col = const.tile([128, 1], f32)
        nc.vector.memset(ones_col[:], 1.0)
        ones_row = const.tile([1, 128], f32)
        nc.vector.memset(ones_row[:], 1.0)
        qsel = const.tile([128, 4], f32)
        nc.vector.memset(qsel[:], 0.0)
        for sub in range(4):
            nc.vector.memset(qsel[32 * sub : 32 * sub + 32, sub : sub + 1], 1.0)
        wrow_d = const.tile([1, H], f32)
        nc.sync.dma_start(out=wrow_d[:], in_=wd[:])
        wrow_q = const.tile([1, H], f32)
        nc.sync.dma_start(out=wrow_q[:], in_=wq[:])
        selt_t = const.tile([128, ntsum, D], f32)
        nc.sync.dma_start(out=selt_t[:], in_=selt[:])
        qbias_t = const.tile([128, 2], f32)
        nc.sync.dma_start(out=qbias_t[:], in_=qbias[:])

        # Broadcast W rows across all 128 partitions via a k=1 matmul.
        wb_d = const.tile([128, H], f32)
        wb_q = const.tile([128, H], f32)
        with tc.tile_pool(name="wbps", bufs=1, space="PSUM") as wbps:
            for wb, wrow in ((wb_d, wrow_d), (wb_q, wrow_q)):
                ps = wbps.tile([128, H], f32, tag="wbps")
                nc.tensor.matmul(
                    ps[:, 0:512], ones_row[:], wrow[:, 0:512], start=True, stop=True
                )
                nc.tensor.matmul(
                    ps[:, 512:H], ones_row[:], wrow[:, 512:H], start=True, stop=True
                )
                nc.scalar.copy(wb[:], ps[:])

        xpool = ctx.enter_context(tc.tile_pool(name="xp", bufs=4))
        spool = ctx.enter_context(tc.tile_pool(name="sp", bufs=4))
        apool = ctx.enter_context(tc.tile_pool(name="apl", bufs=4))
        scrp = ctx.enter_context(tc.tile_pool(name="scr", bufs=2))
        outp = ctx.enter_context(tc.tile_pool(name="outp", bufs=3))
        smallp = ctx.enter_context(tc.tile_pool(name="smallp", bufs=4))
        qpoolp = ctx.enter_context(tc.tile_pool(name="qpl", bufs=2))
        nump = ctx.enter_context(tc.tile_pool(name="nump", bufs=2, space="PSUM"))
        denp = ctx.enter_context(tc.tile_pool(name="denp", bufs=2, space="PSUM"))

        # Score reduction: TT-mult on DVE, then a free-dim reduce either on
        # ACT (activation Copy with accum_out) or DVE (tensor_reduce) —
        # chosen per chunk to balance the two engines.
        def score_col(x_ap, w_ap, accum_ap, name, on_act):
            xw = scrp.tile([128, H], f32, tag="scratch", name=f"xw{name}")
            cnt = x_ap.shape[0]
            nc.vector.tensor_tensor(
                out=xw[0:cnt, :], in0=x_ap, in1=w_ap, op=MULT
            )
            if on_act:
                scr2 = scrp.tile([128, H], f32, tag="scratch2", name=f"s2{name}")
                nc.scalar.activation(
                    scr2[0:cnt, :], xw[0:cnt, :], COPY,
                    bias=0.0, scale=1.0, accum_out=accum_ap,
                )
            else:
                nc.vector.tensor_reduce(
                    out=accum_ap, in_=xw[0:cnt, :],
                    axis=mybir.AxisListType.X, op=ADD,
                )

        def emit_query_batch(b):
            qpack = qpoolp.tile([128, H], f32, tag="qpack", name=f"qpack{b}")
            for sub in range(4):
                nc.sync.dma_start(
                    out=qpack[32 * sub : 32 * sub + 32, :],
                    in_=hidden[4 * b + sub, 1 : 1 + Q, :],
                )
            qscol = smallp.tile([128, 1], f32, tag="qscol", name=f"qscol{b}")
            score_col(qpack[:], wb_q[:], qscol[:], f"q{b}", on_act=True)
            qalpha = smallp.tile([128, 1], f32, tag="qalpha", name=f"qalpha{b}")
            nc.scalar.activation(
                qalpha[:], qscol[:], EXP, bias=qbias_t[:, b : b + 1], scale=1.0
            )
            qat = apool.tile([128, 4], f32, tag="qat", name=f"qat{b}")
            nc.vector.tensor_scalar(
                out=qat[:], in0=qsel[:], scalar1=qalpha[:, 0:1], scalar2=None, op0=MULT
            )
            qnum = nump.tile([4, H], f32, tag="num", name=f"qnum{b}")
            qden = denp.tile([4, 1], f32, tag="den", name=f"qden{b}")
            nc.tensor.matmul(qnum[:, 0:512], qat[:], qpack[:, 0:512], start=True, stop=True)
            nc.tensor.matmul(qnum[:, 512:H], qat[:], qpack[:, 512:H], start=True, stop=True)
            nc.tensor.matmul(qden[:], qat[:], ones_col[:], start=True, stop=True)
            qde = smallp.tile([4, 1], f32, tag="qde", name=f"qde{b}")
            nc.vector.tensor_scalar(
                out=qde[:], in0=qden[:], scalar1=DEN_EPS, scalar2=None, op0=ADD
            )
            qrec = smallp.tile([4, 1], f32, tag="qrec", name=f"qrec{b}")
            nc.vector.reciprocal(qrec[:], qde[:])
            qo = outp.tile([4, H], f32, tag="qo", name=f"qo{b}")
            nc.scalar.activation(qo[:], qnum[:], COPY, bias=0.0, scale=qrec[:, 0:1])
            nc.scalar.dma_start(out=q_out[4 * b : 4 * b + 4, :], in_=qo[:])

        chunk_counter = [0]
        for s in range(SLOTS):
            nt, rem, coff = nts[s], rems[s], coffs[s]
            x = xpool.tile([128, nt, H], f32, tag="x", name=f"x{s}")
            for c in range(nt):
                cnt = 128 if c < nt - 1 else rem
                nc.sync.dma_start(
                    out=x[0:cnt, c, :], in_=hidden[s, 128 * c : 128 * c + cnt, :]
                )
            scol = spool.tile([128, nt], f32, tag="scol", name=f"scol{s}")
            nc.vector.memset(scol[:], NEG_BIAS)
            for c in range(nt):
                cnt = 128 if c < nt - 1 else rem
                score_col(
                    x[0:cnt, c, :], wb_d[0:cnt, :], scol[0:cnt, c : c + 1],
                    f"d{s}_{c}", on_act=(chunk_counter[0] % 3 != 2),
                )
                chunk_counter[0] += 1
            ecol = spool.tile([128, nt], f32, tag="ecol", name=f"ecol{s}")
            nc.scalar.activation(ecol[:], scol[:], EXP, bias=0.0, scale=1.0)
            at = apool.tile([128, nt, D], f32, tag="at", name=f"at{s}")
            for c in range(nt):
                cnt = 128 if c < nt - 1 else rem
                nc.vector.tensor_scalar(
                    out=at[0:cnt, c, :],
                    in0=selt_t[0:cnt, coff + c, :],
                    scalar1=ecol[0:cnt, c : c + 1],
                    scalar2=None,
                    op0=MULT,
                )
            num = nump.tile([D, H], f32, tag="num", name=f"num{s}")
            den = denp.tile([D, 1], f32, tag="den", name=f"den{s}")
            for c in range(nt):
                cnt = 128 if c < nt - 1 else rem
                first = c == 0
                last = c == nt - 1
                nc.tensor.matmul(
                    num[:, 0:512], at[0:cnt, c, :], x[0:cnt, c, 0:512],
                    start=first, stop=last,
                )
                nc.tensor.matmul(
                    num[:, 512:H], at[0:cnt, c, :], x[0:cnt, c, 512:H],
                    start=first, stop=last,
                )
                nc.tensor.matmul(
                    den[:], at[0:cnt, c, :], ones_col[0:cnt, :],
                    start=first, stop=last,
                )
            de = smallp.tile([D, 1], f32, tag="de", name=f"de{s}")
            nc.vector.tensor_scalar(
                out=de[:], in0=den[:], scalar1=DEN_EPS, scalar2=None, op0=ADD
            )
            rec = smallp.tile([D, 1], f32, tag="rec", name=f"rec{s}")
            nc.vector.reciprocal(rec[:], de[:])
            do = outp.tile([D, H], f32, tag="do", name=f"do{s}")
            nc.scalar.activation(do[:], num[:], COPY, bias=0.0, scale=rec[:, 0:1])
            nc.scalar.dma_start(out=doc_out[s, :, :], in_=do[:])

            if s == 0:
                emit_query_batch(0)
            if s == 4:
                emit_query_batch(1)

    nc.compile()
    return nc


def _prepare(query_len, seq_lens):
    """Host-side geometry: spans, slot assignment, selector/bias arrays."""
    ql = np.asarray(query_len).astype(np.int64)
    sl = np.asarray(seq_lens).astype(np.int64)
    offs = ql[:, None] + 2 + np.cumsum(sl, axis=1) - sl  # [B, D] sentence starts
    end = ql + 2 + sl.sum(axis=1)
    span = np.maximum(end, 1 + Q)  # query rows 1..32 must be covered
    order = np.argsort(-span, kind="stable")  # rank -> example id
    slot_spans = tuple(int(span[order[8 * s]]) for s in range(SLOTS))
    nts, rems, coffs = _slot_geometry(slot_spans)
    ntsum = coffs[-1]

    selt_all = np.zeros((NCORES, 128, ntsum, D), np.float32)
    qbias_all = np.full((NCORES, 128, 2), NEG_BIAS, np.float32)
    ex_map = np.empty((NCORES, SLOTS), np.int64)
    for c in range(NCORES):
        for s in range(SLOTS):
            e = int(order[8 * s + c])
            ex_map[c, s] = e
            for j in range(D):
                ln = int(sl[e, j])
                if ln == 0:
                    continue
                o = int(offs[e, j])
                t = np.arange(o, o + ln)
                selt_all[c, t % 128, coffs[s] + t // 128, j] = 1.0
            b, sub = divmod(s, 4)
            qbias_all[c, 32 * sub : 32 * sub + int(ql[e]), b] = 0.0
    return slot_spans, ex_map, selt_all, qbias_all


def kernel(hidden_states, W_doc, b_doc, W_query, b_query, query_len, seq_lens):
    hs = np.ascontiguousarray(np.asarray(hidden_states, dtype=np.float32))
    wd = np.ascontiguousarray(np.asarray(W_doc, np.float32).reshape(1, H))
    wq = np.ascontiguousarray(np.asarray(W_query, np.float32).reshape(1, H))

    slot_spans, ex_map, selt_all, qbias_all = _prepare(query_len, seq_lens)

    nc = _compiled.get(slot_spans)
    if nc is None:
        nc = _build(slot_spans)
        _compiled[slot_spans] = nc

    in_maps = []
    for c in range(NCORES):
        in_maps.append(
            {
                "hidden": np.ascontiguousarray(hs[ex_map[c]]),
                "wd": wd,
                "wq": wq,
                "selt": selt_all[c],
                "qbias": qbias_all[c],
            }
        )

    from concourse.bass_utils import run_bass_kernel_spmd

    res = run_bass_kernel_spmd(nc, in_maps, list(range(NCORES)))

    doc = np.empty((B, D, H), np.float32)
    qp = np.empty((B, H), np.float32)
    for c in range(NCORES):
        r = res.results[c]
        for s in range(SLOTS):
            e = int(ex_map[c, s])
            doc[e] = r["doc_out"][s]
            qp[e] = r["q_out"][s]
    q_bcast = np.broadcast_to(qp[:, None, :], (B, D, H))
    return doc, q_bcast
